# revision 54
# baseline (speedup 1.0000x reference)
"""Trainium2 Bass kernel for the temporal/spatial adapter transformer block.

Sharding: data-parallel over the video batch B=8 -> 1 video (16 frames) per
NeuronCore; all weights replicated. No collectives.

Design (V3; the V2 baseline is kernel_baseline.py):
  - fp8-e4m3 DoubleRow matmuls (2 contraction chunks per pass) for every
    dense GEMM (qkv, proj, fc1, fc2, adapter down/up).  Power-of-two weight
    scales keep fp8 mantissas in range (subnormal-aware), compensated on
    psum evacuation or on the residual-add scalar_tensor_tensor.  Measured
    rel err 1.58e-2 (limit 2e-2); the S branch dominates the error since
    its attention/MLP deltas feed the residual at full magnitude.
  - delta-producing matmuls emit TOKEN-major output (activation tile as the
    stationary operand, weight as the moving operand) so the residual add
    reads psum directly: no fm->token transposes, no delta evacuation.
    Biases enter via a ones-row matmul into the same psum.
  - two ACT table sets only (sigmoid_and_others for erf-gelu adapters +
    sigmoid quick-gelu; exp_and_others for softmax): ~10 table loads per
    invocation instead of ~36.  LN rsqrt via bit-trick + Newton on DVE
    (GpSimd measured ~5x slower than its cost model - avoid).
  - LN gamma/beta folded into consumer weights; v-bias folded into the proj
    bias (softmax rows sum to one); attention q-scale folded into q weights.
  - softmax: merged exp over both k-chunks per head, paired-head row-sum
    banks -> one reciprocal per head pair, 2 broadcast matmuls per chunk,
    psum-shared po for head parities.
  - T/S branch emission staggered by one stage so the ACT-heavy attention
    of one branch overlaps the DVE-heavy MLP of the other; 12-deep residual
    ring overlaps adjacent pair-groups.
  - all weights resident in SBUF (~9 MB fp8); no mid-kernel weight DMA.

Measured (8 cores, repeat-loop differencing): ~1.05-1.10 ms vs 1.89 ms for
the V2 baseline.  TimelineSim predicts 679 us; the gap is DoubleRow
LDWEIGHTS rate and per-instruction dispatch costs the model underestimates.
"""

import sys

import numpy as np
import ml_dtypes

try:
    import concourse.bass  # noqa: F401
except ImportError:  # concourse ships with the container, not on sys.path
    for p in ("/opt/trn_rl_repo", "/root/.axon_site/_ro/trn_rl_repo"):
        if p not in sys.path:
            sys.path.insert(0, p)

import concourse.bass as bass
import concourse.mybir as mybir
import concourse.tile as tile
from concourse import bacc
from concourse.bass_utils import run_bass_kernel_spmd

BF = mybir.dt.bfloat16
F32 = mybir.dt.float32
I32 = mybir.dt.int32
F8 = mybir.dt.float8e4
AF = mybir.ActivationFunctionType
OP = mybir.AluOpType
DR = mybir.MatmulPerfMode.DoubleRow

P = 128
NSEQ = 197          # tokens per frame/sequence
D = 768
DK = D // P         # 6
H = 12
HD = 64
BOT = 192
HID = 4 * D         # 3072
HK = HID // P       # 24
EPS = 1e-5
T = 16              # frames per video
TT = 8              # temporal frames
NCORES = 8
TAU = 2 * NSEQ      # tokens per pair = 394
TAUP = 400          # fp8 tiles padded so chunk strides are 16B-multiples
TAUK = 454          # qkT pad: full-128-col score slices stay in bounds
ROWS = T * NSEQ     # 3152 rows per core

GELU_C = 0.044715
GELU_S = 0.7978845608028654  # sqrt(2/pi)
QK_SCALE = HD ** -0.5
QG = 0.851          # 1.702 / 2 for the tanh form of quick-gelu

ADAPTERS = ("tab", "sa", "ta", "sm", "tm")

bf16 = ml_dtypes.bfloat16
f8e4 = ml_dtypes.float8_e4m3

# fp8 scale choices (powers of two).  stored_weight = scale * true_weight.
S_QKV_K = 64.0   # k,v rows
S_QKV_Q = 512.0  # q rows carry the extra 1/8 attention scale
S_PROJ = 64.0    # proj: stationary for T (evac 1/64) AND moving for S (stt 1/64)
S_AD = 64.0      # adapter down
S_FC1 = 32.0     # bounded so g2 = S_FC1*quickgelu stays under fp8 max (max|qgelu|~4.3)
import os as _os
A_G2 = S_FC1 if int(_os.environ.get("K_G2MODE", "0")) == 0 else 1.0
USESWI = int(_os.environ.get("K_SWI", "0"))    # SwInterleave qk/fc1 weights
S_MOV = 64.0     # token-major delta psum scale (proj / adapter ups)
A_AT = 8.0       # tab-adapter output (attention input) fp8 storage scale
S_FC2P = 2048.0  # fc2 + sm delta psum scale


def _to_f8(a):
    m = np.abs(a).max()
    assert m < 224.0, f"fp8 overflow risk: max {m}"
    return a.astype(f8e4)


# ----------------------------------------------------------------------------
# host-side weight preprocessing (shared by all cores)
# ----------------------------------------------------------------------------

def fm(mat):  # [out, in] -> [128, in//128, out]  (lhsT / moving layout)
    o, i = mat.shape
    return np.ascontiguousarray(mat.T.reshape(i // P, P, o).transpose(1, 0, 2))


def swpack(wfm, nk, nblk):
    """[128, nk, nblk*128] fp8 fm layout -> [128, nk//2, nblk, 2, 128]
    DoubleRowSwInterleave packing (A/B pairs interleaved, columns
    reversed, per 2-chunk x 128-col block)."""
    p = wfm.shape[0]
    out = np.zeros((p, nk // 2, nblk, 2 * P), wfm.dtype)
    for kp in range(nk // 2):
        for cb in range(nblk):
            A = wfm[:, 2 * kp, cb * P:(cb + 1) * P]
            B = wfm[:, 2 * kp + 1, cb * P:(cb + 1) * P]
            out[:, kp, cb, 0::2] = A[:, ::-1]
            out[:, kp, cb, 1::2] = B[:, ::-1]
    return out.reshape(p, nk // 2, nblk, 2, P)


def preprocess_weights(inp):
    w = {}
    g1 = np.asarray(inp["n1_g"], np.float32); b1 = np.asarray(inp["n1_b"], np.float32)
    g2 = np.asarray(inp["n2_g"], np.float32); b2 = np.asarray(inp["n2_b"], np.float32)

    # ---- qkv (LN1 folded; q rows scaled; v bias folded into proj bias)
    qkv_raw = np.asarray(inp["qkv_w"], np.float32)
    qkv = qkv_raw * g1[None, :]
    qkv_b = qkv_raw @ b1                                       # [2304]
    qkv_s = qkv.copy()
    qkv_s[:D] *= QK_SCALE * S_QKV_Q
    qkv_s[D:] *= S_QKV_K
    wqkv_full = _to_f8(fm(qkv_s))                              # [128, 6, 2304]
    if USESWI:
        w["wqk_sw"] = swpack(wqkv_full[:, :, :2 * D].copy(), DK, 2 * DK)
        w["wqkv"] = np.ascontiguousarray(wqkv_full[:, :, 2 * D:])  # v only
    else:
        w["wqkv"] = wqkv_full
    bqk = np.zeros((P, 2 * DK), np.float32)
    bqk[:, :DK] = (qkv_b[:D] * QK_SCALE).reshape(DK, P).T
    bqk[:, DK:] = qkv_b[D:2 * D].reshape(DK, P).T
    w["bqk"] = bqk                                             # [128, 12]

    # ---- proj (stationary for T, moving for S; same layout & scale)
    proj_w = np.asarray(inp["proj_w"], np.float32)
    proj_b = np.asarray(inp["proj_b"], np.float32) + proj_w @ qkv_b[2 * D:]
    w["wproj"] = _to_f8(fm(proj_w * S_PROJ))                   # [128, 6, 768]
    w["bproj"] = proj_b.reshape(DK, P).T.copy()

    # ---- fc1 (LN2 folded)
    fc1_raw = np.asarray(inp["fc1_w"], np.float32)
    fc1_w = fc1_raw * g2[None, :]
    fc1_b = np.asarray(inp["fc1_b"], np.float32) + fc1_raw @ b2
    wfc1_full = _to_f8(fm(fc1_w * S_FC1))                      # [128, 6, 3072]
    if USESWI:
        w["wfc1_sw"] = swpack(wfc1_full, DK, HK)
    else:
        w["wfc1"] = wfc1_full
    w["bfc1t"] = (1.702 * fc1_b).reshape(HK, P).T.copy()       # sigmoid bias
    w["bfc1s"] = (S_FC1 * fc1_b).reshape(HK, P).T.copy()       # stt scalar
    w["bfc1m"] = fc1_b.reshape(HK, P).T.copy()                 # u_m bias

    # ---- fc2: psum = S_FC2P * (fc2_w @ qgelu) given g2 carries A_G2*qgelu
    fc2_w = np.asarray(inp["fc2_w"], np.float32)
    w["wfc2"] = _to_f8(fm(fc2_w * (S_FC2P / A_G2)))            # [128, 24, 768]
    w["bfc2"] = np.asarray(inp["fc2_b"], np.float32).reshape(DK, P).T.copy()

    # ---- adapters
    for ad in ADAPTERS:
        dw = np.asarray(inp[ad + "_dw"], np.float32)
        db = np.asarray(inp[ad + "_db"], np.float32)
        uw = np.asarray(inp[ad + "_uw"], np.float32)
        if ad in ("tab", "sa"):   # consume LN1 output
            dwe = dw * g1[None, :]; dbe = db + dw @ b1
        elif ad == "sm":          # consumes LN2 output
            dwe = dw * g2[None, :]; dbe = db + dw @ b2
        else:                     # ta, tm consume attn/mlp outputs
            dwe = dw; dbe = db
        w["w%sd" % ad] = _to_f8(fm(dwe * S_AD))                # [128, 6, 192]
        bd = np.zeros((P, 2), np.float32)
        bd[:, 0] = dbe[:P]
        bd[:64, 1] = dbe[P:]
        w["b%sd" % ad] = bd
        # up weights, [bot-part, out] layout, zero rows 64:128 of chunk 1.
        # gs tiles carry 2*gelu; psum scale S_MOV (S_FC2P when sharing the
        # fc2 psum, i.e. sm).
        sc = (S_FC2P if ad == "sm" else S_MOV) * 0.5
        up = np.zeros((2 * P, D), np.float32)
        up[:BOT] = sc * uw.T
        w["w%su" % ad] = _to_f8(up.reshape(2, P, D).transpose(1, 0, 2))

    w["btabu_s"] = (A_AT * np.asarray(inp["tab_ub"], np.float32)
                    ).reshape(DK, P).T.copy()

    # ---- bias rows for token-major delta psums (pre-scaled by psum factor)
    ta_ub = np.asarray(inp["ta_ub"], np.float32)
    tm_ub = np.asarray(inp["tm_ub"], np.float32)
    sa_ub = np.asarray(inp["sa_ub"], np.float32)
    sm_ub = np.asarray(inp["sm_ub"], np.float32)
    w["brT1"] = (S_MOV * ta_ub).reshape(1, D).astype(bf16)
    w["brT2"] = (S_MOV * tm_ub).reshape(1, D).astype(bf16)
    w["brS1"] = (S_MOV * (proj_b + sa_ub)).reshape(1, D).astype(bf16)
    w["brS2"] = (S_FC2P * (np.asarray(inp["fc2_b"], np.float32) + sm_ub)
                 ).reshape(1, D).astype(bf16)

    w["ident"] = np.eye(P, dtype=bf16)
    w["onesc"] = np.ones((P, P), dtype=bf16)   # sum matmuls / bias rows
    ones2 = np.zeros((2, P), np.float32)
    ones2[0, :HD] = 1.0
    ones2[1, HD:] = 1.0
    w["ones2"] = ones2.astype(bf16)            # 2-head broadcast lhsT
    return w


WEIGHT_SPECS = [
    ("bqk", [P, 2 * DK], F32),
] + ([
    ("wqkv", [P, DK, D], F8),
    ("wqk_sw", [P, DK // 2, 2 * DK, 2, P], F8),
    ("wfc1_sw", [P, DK // 2, HK, 2, P], F8),
] if USESWI else [
    ("wqkv", [P, DK, 3 * D], F8),
    ("wfc1", [P, DK, HID], F8),
]) + [
    ("wproj", [P, DK, D], F8), ("bproj", [P, DK], F32),
    ("bfc1t", [P, HK], F32), ("bfc1s", [P, HK], F32),
    ("bfc1m", [P, HK], F32),
    ("wfc2", [P, HK, D], F8), ("bfc2", [P, DK], F32),
    ("btabu_s", [P, DK], F32),
    ("brT1", [1, D], BF), ("brT2", [1, D], BF),
    ("brS1", [1, D], BF), ("brS2", [1, D], BF),
    ("ident", [P, P], BF), ("onesc", [P, P], BF), ("ones2", [2, P], BF),
] + [
    it for ad in ADAPTERS for it in [
        ("w%sd" % ad, [P, DK, BOT], F8),
        ("b%sd" % ad, [P, 2], F32),
        ("w%su" % ad, [P, 2, D], F8),
    ]
]


# ----------------------------------------------------------------------------
# program emission
# ----------------------------------------------------------------------------

# token tiles of a pair: (row_offset_within_pair, nrows, fm_col_offset)
PAIR_TILES = [(0, P, 0), (P, NSEQ - P, P),
              (NSEQ, P, NSEQ), (NSEQ + P, NSEQ - P, NSEQ + P)]
# 384-column halves of the feature dim for token-major delta matmuls
DHALVES = ((0, 384), (384, 384))

GS_BUFS = 3

import os
PSA = int(os.environ.get("K_PSA", "3"))
PSS = int(os.environ.get("K_PSS", "2"))
PST = int(os.environ.get("K_PST", "2"))
PSD = int(os.environ.get("K_PSD", "1"))
XRES = int(os.environ.get("K_XRES", "12"))
VT = int(os.environ.get("K_VT", "4"))
U2B = int(os.environ.get("K_U2", "2"))
THB = int(os.environ.get("K_TH", "2"))
AEB = int(os.environ.get("K_AE", "2"))
XNT = int(os.environ.get("K_XNT", "2"))
G2MODE = int(os.environ.get("K_G2MODE", "0"))  # 0=DVE stt, 1=ACT-u + Pool TT
STAGGER = int(os.environ.get("K_STAGGER", "1"))  # stages to advance T before S
NEWTON = int(os.environ.get("K_NEWTON", "1"))
POOLLN = int(os.environ.get("K_POOLLN", "0"))  # LN eps/xn-apply on GpSimd
USEDR = int(os.environ.get("K_DR", "1"))       # DoubleRow fp8 matmuls


class Ctx:
    pass


def make_pools(ctx, tc, es):
    def pool(name, bufs):
        return es.enter_context(tc.tile_pool(name=name, bufs=bufs))

    def ppool(name, bufs):
        return es.enter_context(tc.tile_pool(name=name, bufs=bufs, space="PSUM"))

    ctx.weights = pool("weights", 1)
    ctx.xres = pool("xres", XRES)    # token-major f32 residual stream
    ctx.small = pool("small", 7)     # bn stats, newton scratch
    ctx.xn = pool("xn", 2)           # token-major bf16 LN output
    ctx.xnT = pool("xnT", XNT)         # fp8 feature-major LN output
    ctx.qk = pool("qk", 2)           # q,k feature-major bf16
    ctx.vt = pool("vt", VT)           # v token-major bf16
    ctx.ae = pool("ae", AEB)           # exp'd scores bf16
    ctx.rb = pool("rb", 2)           # per-seq softmax recips [12,197]
    ctx.oTu = pool("oTu", 2)         # unnormalized o^T bf16
    ctx.oT = pool("oT", 2)           # normalized o^T fp8
    ctx.fmB = pool("fmB", 2)         # attnT / mlpT fp8
    ctx.g2 = pool("g2", 2)           # mlp gelu output fp8
    ctx.gs = pool("gs", GS_BUFS)     # adapter gelu output fp8 [128,2,TAUP]
    ctx.u = pool("u", 3)             # adapter u bf16
    ctx.u2 = pool("u2", U2B)           # adapter scratch bf16
    ctx.th = pool("th", THB)           # tanh scratch bf16

    ctx.psA = ppool("psA", PSA)      # generic matmul outputs [128, 512] f32
    ctx.psS = ppool("psS", PSS)      # attention scores (+row-sums)
    ctx.psT = ppool("psT", PST)      # transposes [128, 512] bf16
    ctx.psD = ppool("psD", PSD)      # token-major delta psums


def load_weights(ctx, nc, d):
    ctx.W = {}
    for name, shape, dt in WEIGHT_SPECS:
        t = ctx.weights.tile(shape, dt, tag=name)
        nc.sync.dma_start(t[:], d[name][:])
        ctx.W[name] = t



def f8tile(ctx, nc, pool, nch, tag):
    """fp8 [P, nch, TAUP] tile with pad columns zeroed (DoubleRow moving
    operands read the full padded width; stale bytes could be NaN)."""
    t = pool.tile([P, nch, TAUP], F8, tag=tag, name=tag)
    nc.vector.memset(t[:, :, TAU:TAUP], 0.0)
    return t


def new_ps(ctx, name="mmps"):
    return ctx.psA.tile([P, 512], F32, tag="mm", name=name)


MAGIC = 0x5f3759df


def emit_rsqrt(ctx, nc, out, ve, pi):
    """out[:pi,0:1] = 1/sqrt(ve[:pi,0:1]) via bit trick + 2 Newton iters."""
    sm = ctx.small
    y = sm.tile([P, 4], F32, tag="nt_y")
    h = sm.tile([P, 1], I32, tag="nt_h")
    v = nc.gpsimd if POOLLN else nc.vector
    nc.vector.tensor_scalar(h[:pi], ve[:pi].bitcast(I32), 1, None,
                            op0=OP.logical_shift_right)
    nc.vector.tensor_scalar(y[:pi, 0:1].bitcast(I32), h[:pi], -1, MAGIC,
                            op0=OP.mult, op1=OP.add)
    for it in range(1, NEWTON + 1):
        v.tensor_tensor(y[:pi, 1:2], y[:pi, 0:1], y[:pi, 0:1], op=OP.mult)
        v.tensor_tensor(y[:pi, 2:3], y[:pi, 1:2], ve[:pi], op=OP.mult)
        v.tensor_scalar(y[:pi, 3:4], y[:pi, 2:3], -0.5, 1.5,
                        op0=OP.mult, op1=OP.add)
        dst = out[:pi, 0:1] if it == NEWTON else y[:pi, 0:1]
        v.tensor_tensor(dst, y[:pi, 0:1], y[:pi, 3:4], op=OP.mult)


def emit_ln(ctx, nc, xts, tiles, alt):
    """token-major LN on xts (f32) -> feature-major fp8 [128, DK, TAUP].
    gamma/beta are folded into consumer weights, so the transpose evac is a
    plain cast.  alt: 0/1 picks which engine takes the even evacs."""
    W = ctx.W

    def cp_dve(dst, src):
        nc.vector.tensor_copy(dst, src)

    def cp_act(dst, src):
        nc.scalar.copy(dst, src)

    evac = (cp_dve, cp_act) if alt == 0 else (cp_act, cp_dve)
    xns = []
    for i, (r0, pi, co) in enumerate(tiles):
        xt = xts[i]
        st = ctx.small.tile([P, 2, 6], F32, tag="bnst")
        nc.vector.bn_stats(st[:pi, 0, :], xt[:pi, 0:D // 2])
        nc.vector.bn_stats(st[:pi, 1, :], xt[:pi, D // 2:D])
        mv = ctx.small.tile([P, 3], F32, tag="bnmv")
        nc.vector.bn_aggr(mv[:pi, 0:2], st[:pi])
        (nc.gpsimd if POOLLN else nc.vector).tensor_scalar(
            mv[:pi, 2:3], mv[:pi, 1:2], EPS, None, op0=OP.add)
        rstd = ctx.small.tile([P, 1], F32, tag="rstd")
        emit_rsqrt(ctx, nc, rstd, mv[:, 2:3], pi)
        negmr = ctx.small.tile([P, 1], F32, tag="negmr")
        nc.vector.scalar_tensor_tensor(negmr[:pi], mv[:pi, 0:1], -1.0,
                                       rstd[:pi], op0=OP.mult, op1=OP.mult)
        xn = ctx.xn.tile([P, D], BF, tag="xn")
        nc.scalar.activation(xn[:pi], xt[:pi], AF.Identity,
                             bias=negmr[:pi], scale=rstd[:pi])
        xns.append(xn)
    xnT = f8tile(ctx, nc, ctx.xnT, DK, "xnT")
    for half in range(2):
        i0 = 2 * half
        (r0a, pa, coa), (r0b, pb, cob) = tiles[i0], tiles[i0 + 1]
        for j in range(DK):
            tp = ctx.psT.tile([P, 512], BF, tag="tp", name="tp")
            nc.tensor.transpose(tp[:P, :pa], xns[i0][:pa, j * P:(j + 1) * P],
                                W["ident"][:pa, :pa])
            nc.tensor.transpose(tp[:P, P:P + pb], xns[i0 + 1][:pb, j * P:(j + 1) * P],
                                W["ident"][:pb, :pb])
            evac[(i0 * DK + j) % 2](xnT[:, j, coa:coa + NSEQ], tp[:, :NSEQ])
    return xnT


def mm_chain(nc, out, wsel, insel, nk, trailing=False):
    """Accumulation chain over nk contraction chunks; DoubleRow pairs when
    USEDR.  wsel/insel map a (lo, hi) chunk slice to the operand APs."""
    if USEDR:
        for k in range(nk // 2):
            nc.tensor.matmul(out, wsel(2 * k, 2 * k + 2), insel(2 * k, 2 * k + 2),
                             start=(k == 0), stop=(not trailing) and k == nk // 2 - 1,
                             perf_mode=DR)
    else:
        for k in range(nk):
            nc.tensor.matmul(out, wsel(k, k + 1), insel(k, k + 1),
                             start=(k == 0), stop=(not trailing) and k == nk - 1)


def emit_fm_dr(ctx, nc, wap, inT, nout, combine, nk=DK):
    """feature-major fp8 matmul (weights stationary, activations moving).
    combine(mc, ps) consumes each [128, TAUP] psum."""
    for mc in range(nout):
        ps = new_ps(ctx)
        mm_chain(nc, ps[:, :TAUP],
                 lambda a, b, mc=mc: wap[:, a:b, mc * P:(mc + 1) * P],
                 lambda a, b: inT[:, a:b, :], nk)
        combine(mc, ps)


def emit_adapter(ctx, nc, ad, inT):
    """adapter down + tanh-gelu; returns gs fp8 [128, 2, TAUP] carrying
    2*gelu(down(x)).  rows 64:128 of chunk 1 stay zero (startup memset)."""
    W = ctx.W
    wd, bd = W["w%sd" % ad], W["b%sd" % ad]
    gs = ctx.gs.tile([P, 2, TAUP], F8, tag="gs")
    nc.vector.memset(gs[:], 0.0)
    for oc, (ob, osz) in enumerate(((0, P), (P, 64))):
        ps = new_ps(ctx)
        mm_chain(nc, ps[:osz, :TAUP],
                 lambda a, b, ob=ob, osz=osz: wd[:, a:b, ob:ob + osz],
                 lambda a, b: inT[:, a:b, :], DK)
        u = ctx.u.tile([P, TAU], BF, tag="u")
        nc.scalar.activation(u[:osz], ps[:osz, :TAU], AF.Identity,
                             bias=bd[:osz, oc:oc + 1], scale=1.0 / S_AD)
        th = ctx.th.tile([P, TAU], BF, tag="thad")
        nc.scalar.activation(th[:osz], u[:osz], AF.Erf, scale=2.0 ** -0.5)
        nc.vector.scalar_tensor_tensor(gs[:osz, oc, :TAU], th[:osz], 1.0,
                                       u[:osz], op0=OP.add, op1=OP.mult)
    return gs


def emit_attention(ctx, nc, inT, tiles, alpha=1.0, out_fp8=True):
    """multi-head attention core on fp8 feature-major input inT (stored as
    alpha*x).  Returns (oT fp8 or None, oTu bf16); with out_fp8=False the
    normalization is applied in place on the bf16 oTu (extra precision for
    the S branch whose delta is a full attention output)."""
    W = ctx.W
    wq = W["wqkv"]
    # q,k feature-major bf16
    qkT = ctx.qk.tile([P, 2 * DK, TAUK], BF, tag="qkT")
    nc.vector.memset(qkT[:, :, TAU:TAUK], 0.0)
    for oc in range(2 * DK):
        ps = new_ps(ctx)
        if USESWI and USEDR:
            for kp in range(DK // 2):
                nc.tensor.matmul(ps[:, :TAUP], W["wqk_sw"][:, kp, oc, :, :],
                                 inT[:, 2 * kp:2 * kp + 2, :], start=(kp == 0),
                                 stop=(kp == DK // 2 - 1),
                                 perf_mode=mybir.MatmulPerfMode.DoubleRowSwInterleave)
        else:
            mm_chain(nc, ps[:, :TAUP],
                     lambda a, b, oc=oc: wq[:, a:b, oc * P:(oc + 1) * P],
                     lambda a, b: inT[:, a:b, :], DK)
        sc = 1.0 / ((S_QKV_Q if oc < DK else S_QKV_K) * alpha)
        if oc % 2 == 0:
            nc.scalar.activation(qkT[:, oc, :TAU], ps[:, :TAU], AF.Identity,
                                 bias=W["bqk"][:, oc:oc + 1], scale=sc)
        else:
            nc.vector.tensor_scalar(qkT[:, oc, :TAU], ps[:, :TAU], sc,
                                    W["bqk"][:, oc:oc + 1],
                                    op0=OP.mult, op1=OP.add)
    # v token-major bf16
    vts = []
    for i, (r0, pi, co) in enumerate(tiles):
        vt = ctx.vt.tile([P, D], BF, tag="vtok")
        for nb, nsz in ((0, 512), (512, 256)):
            vof = 0 if USESWI else 2 * D
            ps = new_ps(ctx, name="psv")
            mm_chain(nc, ps[:pi, :nsz],
                     lambda a, b, co=co, pi=pi: inT[:, a:b, co:co + pi],
                     lambda a, b, nb=nb, nsz=nsz: wq[:, a:b, vof + nb:vof + nb + nsz],
                     DK)
            nc.scalar.activation(vt[:pi, nb:nb + nsz], ps[:pi, :nsz], AF.Copy,
                                 scale=1.0 / (S_QKV_K * alpha))
        vts.append(vt)

    oTu = ctx.oTu.tile([P, DK, TAU], BF, tag="oTu")
    oT = f8tile(ctx, nc, ctx.oT, DK, "oT") if out_fp8 else None
    kts = ((0, P), (P, NSEQ - P))
    for j in range(2):  # seq in pair
        c0 = j * NSEQ
        rb2 = ctx.rb.tile([1, 2, DK, NSEQ], BF, tag="rb2")
        for hp in range(H // 2):   # head pairs (2*hp, 2*hp+1)
            qch, kch = hp, DK + hp
            sT0 = None
            aes = []
            for h in (2 * hp, 2 * hp + 1):
                qof = HD * (h % 2)
                q = qkT[qof:qof + HD, qch, c0:c0 + NSEQ]
                sT = ctx.psS.tile([P, 2, 256], F32, tag="sT", name="sT")
                for kt, (kb, kp) in enumerate(kts):
                    # full-128 stationary slice: rows kp:128 of chunk 1 get
                    # junk (finite) scores, never consumed downstream.
                    nc.tensor.matmul(sT[:, kt, :NSEQ],
                                     qkT[qof:qof + HD, kch, c0 + kb:c0 + kb + P],
                                     q, start=True, stop=True)
                ae = ctx.ae.tile([P, 2, NSEQ], BF, tag="ae")
                nc.scalar.activation(ae[:, :, :], sT[:, :, :NSEQ], AF.Exp)
                if h % 2 == 0:
                    sT0 = sT
                # row-sums into the even head's scores bank (dead post-exp)
                for kt, (kb, kp) in enumerate(kts):
                    nc.tensor.matmul(sT0[0:1, h % 2, :NSEQ],
                                     W["onesc"][:kp, 0:1],
                                     ae[:kp, kt, :], start=(kt == 0), stop=(kt == 1))
                aes.append(ae)
            # one reciprocal for the pair
            with nc.allow_low_precision(reason="softmax recip in bf16"):
                nc.vector.reciprocal(rb2[0:1, :, qch, :],
                                     sT0[0:1, :, :NSEQ])
            # shared po psum: even head rows 0:64, odd head rows 64:128
            po = new_ps(ctx, name="po")
            for h in (2 * hp, 2 * hp + 1):
                qof = HD * (h % 2)
                for kt, (kb, kp) in enumerate(kts):
                    nc.tensor.matmul(po[qof:qof + HD, :NSEQ],
                                     vts[2 * j + kt][:kp, h * HD:(h + 1) * HD],
                                     aes[h % 2][:kp, kt, :],
                                     start=(kt == 0), stop=(kt == 1))
            if hp % 2 == 0:
                nc.scalar.copy(oTu[:, qch, c0:c0 + NSEQ], po[:, :NSEQ])
            else:
                nc.vector.tensor_copy(oTu[:, qch, c0:c0 + NSEQ], po[:, :NSEQ])
        # normalize: broadcast 2 heads per chunk, multiply into fp8 oT
        for ch in range(DK):
            bc = new_ps(ctx, name="bc")
            nc.tensor.matmul(bc[0:HD, :NSEQ], W["onesc"][0:1, :HD],
                             rb2[0:1, 0, ch, :], start=True, stop=True)
            nc.tensor.matmul(bc[HD:P, :NSEQ], W["onesc"][0:1, :HD],
                             rb2[0:1, 1, ch, :], start=True, stop=True)
            dst = oT if out_fp8 else oTu
            nc.vector.tensor_tensor(dst[:, ch, c0:c0 + NSEQ],
                                    oTu[:, ch, c0:c0 + NSEQ], bc[:, :NSEQ],
                                    op=OP.mult)
    return oT, oTu


def emit_delta_tok(ctx, nc, stat, wmov, brow, xts, tiles, comp, nk,
                   extra=None, dr=True):
    """token-major delta: psum accumulates stationary-activation x
    moving-weight matmuls plus a bias row; a scalar_tensor_tensor folds
    comp*psum into the f32 residual in place.  dr=False runs plain (non
    DoubleRow) matmuls so the stationary may be bf16 for extra precision."""
    W = ctx.W
    for i, (r0, pi, co) in enumerate(tiles):
        for hf, (h0, hw) in enumerate(DHALVES):
            ps = ctx.psD.tile([P, 512], F32, tag="dl", name="dl")
            if dr:
                mm_chain(nc, ps[:pi, :hw],
                         lambda a, b, co=co, pi=pi: stat[:, a:b, co:co + pi],
                         lambda a, b, h0=h0, hw=hw: wmov[:, a:b, h0:h0 + hw],
                         nk, trailing=True)
            else:
                for k in range(nk):
                    nc.tensor.matmul(ps[:pi, :hw], stat[:, k, co:co + pi],
                                     wmov[:, k, h0:h0 + hw],
                                     start=(k == 0), stop=False)
            if extra is not None:
                stat2, wmov2, nk2 = extra
                if USEDR:
                    for k in range(nk2 // 2):
                        nc.tensor.matmul(ps[:pi, :hw],
                                         stat2[:, 2 * k:2 * k + 2, co:co + pi],
                                         wmov2[:, 2 * k:2 * k + 2, h0:h0 + hw],
                                         start=False, stop=False, perf_mode=DR)
                else:
                    for k in range(nk2):
                        nc.tensor.matmul(ps[:pi, :hw],
                                         stat2[:, k, co:co + pi],
                                         wmov2[:, k, h0:h0 + hw],
                                         start=False, stop=False)
            nc.tensor.matmul(ps[:pi, :hw], W["onesc"][0:1, :pi],
                             brow[0:1, h0:h0 + hw], start=False, stop=True)
            nc.vector.scalar_tensor_tensor(
                xts[i][:pi, h0:h0 + hw], ps[:pi, :hw], comp,
                xts[i][:pi, h0:h0 + hw], op0=OP.mult, op1=OP.add)


def emit_mlp_g2(ctx, nc, xn2T):
    """fc1 + quick-gelu -> g2 fp8 [128, HK, TAUP] = A_G2*qgelu. Generator:
    yields after half the columns for finer T/S interleave."""
    W = ctx.W
    g2 = f8tile(ctx, nc, ctx.g2, HK, "g2")
    for oc in range(HK):
        ps = new_ps(ctx)
        if USESWI and USEDR:
            for kp in range(DK // 2):
                nc.tensor.matmul(ps[:, :TAUP], W["wfc1_sw"][:, kp, oc, :, :],
                                 xn2T[:, 2 * kp:2 * kp + 2, :], start=(kp == 0),
                                 stop=(kp == DK // 2 - 1),
                                 perf_mode=mybir.MatmulPerfMode.DoubleRowSwInterleave)
        else:
            mm_chain(nc, ps[:, :TAUP],
                     lambda a, b, oc=oc: W["wfc1"][:, a:b, oc * P:(oc + 1) * P],
                     lambda a, b: xn2T[:, a:b, :], DK)
        sg = ctx.th.tile([P, TAU], BF, tag="thm")
        nc.scalar.activation(sg[:], ps[:, :TAU], AF.Sigmoid, scale=1.702 / S_FC1,
                             bias=W["bfc1t"][:, oc:oc + 1])
        if G2MODE == 1:
            um = ctx.u.tile([P, TAU], BF, tag="um")
            nc.scalar.activation(um[:], ps[:, :TAU], AF.Identity,
                                 bias=W["bfc1m"][:, oc:oc + 1], scale=1.0 / S_FC1)
            nc.gpsimd.tensor_tensor(g2[:, oc, :TAU], um[:], sg[:], op=OP.mult)
        else:
            nc.vector.scalar_tensor_tensor(g2[:, oc, :TAU], ps[:, :TAU],
                                           W["bfc1s"][:, oc:oc + 1], sg[:],
                                           op0=OP.add, op1=OP.mult)
    return g2


def emit_pair_gen(ctx, nc, d, branch, rowbase):
    W = ctx.W
    tiles = PAIR_TILES
    # ---- stage A: load + LN1
    xts = []
    for (r0, pi, co) in tiles:
        xt = ctx.xres.tile([P, D], F32, tag="xres")
        nc.sync.dma_start(xt[:pi], d["x"][bass.ds(rowbase + r0, pi), :])
        xts.append(xt)
    xnT = emit_ln(ctx, nc, xts, tiles, alt=0)
    yield

    # ---- branch-specific pre-attention adapter
    if branch == "T":
        gs_tab = emit_adapter(ctx, nc, "tab", xnT)
        aT = f8tile(ctx, nc, ctx.xnT, DK, "aT")

        def tab_comb(mc, ps):
            nc.scalar.activation(aT[:, mc, :TAU], ps[:, :TAU], AF.Identity,
                                 bias=W["btabu_s"][:, mc:mc + 1],
                                 scale=A_AT / S_MOV)
        emit_fm_dr(ctx, nc, W["wtabu"], gs_tab, DK, tab_comb, nk=2)
        attn_in = aT
        gs_sa = None
    else:
        gs_sa = emit_adapter(ctx, nc, "sa", xnT)
        attn_in = xnT
    yield

    # ---- attention
    oT, oTu = emit_attention(ctx, nc, attn_in, tiles,
                             alpha=A_AT if branch == "T" else 1.0,
                             out_fp8=(branch == "T"))
    yield

    # ---- proj + delta1 + first residual
    if branch == "T":
        attnT = f8tile(ctx, nc, ctx.fmB, DK, "fmB")

        def proj_comb(mc, ps):
            nc.scalar.activation(attnT[:, mc, :TAU], ps[:, :TAU], AF.Identity,
                                 bias=W["bproj"][:, mc:mc + 1], scale=1.0 / S_PROJ)
        emit_fm_dr(ctx, nc, W["wproj"], oT, DK, proj_comb)
        gs_ta = emit_adapter(ctx, nc, "ta", attnT)
        emit_delta_tok(ctx, nc, gs_ta, W["wtau"], W["brT1"], xts, tiles,
                       1.0 / S_MOV, 2)
    else:
        emit_delta_tok(ctx, nc, oTu, W["wproj"], W["brS1"], xts, tiles,
                       1.0 / S_MOV, DK, extra=(gs_sa, W["wsau"], 2), dr=False)
    yield

    # ---- LN2
    xn2T = emit_ln(ctx, nc, xts, tiles, alt=1)
    yield

    # ---- MLP
    g2 = emit_mlp_g2(ctx, nc, xn2T)
    if branch == "S":
        gs_sm = emit_adapter(ctx, nc, "sm", xn2T)
    yield

    if branch == "T":
        mlpT = f8tile(ctx, nc, ctx.fmB, DK, "fmB")

        def fc2_comb(mc, ps):
            nc.scalar.activation(mlpT[:, mc, :TAU], ps[:, :TAU], AF.Identity,
                                 bias=W["bfc2"][:, mc:mc + 1], scale=1.0 / S_FC2P)
        emit_fm_dr(ctx, nc, W["wfc2"], g2, DK, fc2_comb, nk=HK)
        gs_tm = emit_adapter(ctx, nc, "tm", mlpT)
        emit_delta_tok(ctx, nc, gs_tm, W["wtmu"], W["brT2"], xts, tiles,
                       1.0 / S_MOV, 2)
    else:
        emit_delta_tok(ctx, nc, g2, W["wfc2"], W["brS2"], xts, tiles,
                       1.0 / S_FC2P, HK, extra=(gs_sm, W["wsmu"], 2))

    # ---- store
    for i, (r0, pi, co) in enumerate(tiles):
        nc.sync.dma_start(d["y"][bass.ds(rowbase + r0, pi), :], xts[i][:pi, :])


def build_program(npairs=4, loop=False, reps=1):
    import contextlib
    nc = bacc.Bacc("TRN2", target_bir_lowering=False, debug=False,
                   num_devices=NCORES)
    d = {}
    d["x"] = nc.dram_tensor("x", [ROWS, D], F32, kind="ExternalInput").ap()
    for name, shape, dt in WEIGHT_SPECS:
        d[name] = nc.dram_tensor(name, shape, dt, kind="ExternalInput").ap()
    d["y"] = nc.dram_tensor("y", [ROWS, D], F32, kind="ExternalOutput").ap()

    with tile.TileContext(nc) as tc:
        with contextlib.ExitStack() as es:
            ctx = Ctx()
            make_pools(ctx, tc, es)
            load_weights(ctx, nc, d)

            def body_pairgroup(i):
                gens = [emit_pair_gen(ctx, nc, d, "T", i),
                        emit_pair_gen(ctx, nc, d, "S", i + TT * NSEQ)]
                done = [False, False]
                for _ in range(STAGGER):
                    try:
                        next(gens[0])
                    except StopIteration:
                        done[0] = True
                while not all(done):
                    for gi, g in enumerate(gens):
                        if not done[gi]:
                            try:
                                next(g)
                            except StopIteration:
                                done[gi] = True

            def body_all():
                if loop:
                    with tc.For_i(0, npairs * TAU, TAU, staggered_reset=True) as i:
                        body_pairgroup(i)
                else:
                    for p in range(npairs):
                        body_pairgroup(p * TAU)

            if reps > 1:
                with tc.For_i(0, reps, 1):
                    body_all()
            else:
                body_all()
    nc.compile()
    return nc


# ----------------------------------------------------------------------------
# harness entry point
# ----------------------------------------------------------------------------

_CACHED = {}


def kernel(**inputs):
    if "nc" not in _CACHED:
        _CACHED["nc"] = build_program()
    nc = _CACHED["nc"]
    w = preprocess_weights(inputs)
    x = np.asarray(inputs["x"], np.float32)  # [128, 197, 768]
    in_maps = []
    for c in range(NCORES):
        m = dict(w)
        m["x"] = np.ascontiguousarray(
            x[c * T:(c + 1) * T].reshape(ROWS, D))
        in_maps.append(m)
    res = run_bass_kernel_spmd(nc, in_maps, core_ids=list(range(NCORES)))
    out = np.stack([r["y"].reshape(T, NSEQ, D) for r in res.results])
    return out.reshape(NCORES * T, NSEQ, D)
